# revision 29
# baseline (speedup 1.0000x reference)
"""Trainium2 Bass kernel: 3x3 conv2d (stride 1, pad 1), NCHW.

x (32, 64, 112, 112) f32, weight (1, 128, 64, 3, 3) f32 -> out (32, 128, 112, 112) f32.

Strategy: data-parallel over batch across 8 cores (4 images/core).
Per core, conv is computed as PSUM-accumulating matmuls: x is host-padded to
(114, 114) so each tap's shifted input window is a constant free-dim offset
into the flat [in_c=64, 114*114] SBUF image. Output is produced in padded
row-major (112 x 114) layout and sliced on the host.

Active variant (pack5x_fp16 = pack5 + 4-deep input prefetch): 5 all-K=128
fp16 matmuls per 512-wide output
block - the minimum possible, since each K=128 matmul can contract at most 2
of the 9 taps (C=64): tile A = [x; x>>WP] (host-prepped, one DMA) covers tap
pairs (0,d)+(1,d) at offsets j0+d and the zero-padded single (2,1) at
j0+WP+1; tile B = [x; x>>2] (built on-device from A by two same-partition
DVE copies) covers (2,0)+(2,2) at j0+2*WP. PSUM is drained to fp16 on the
Activation engine; the x DMA is column-chunked so the first matmuls start
~2.5us in; out-DMA groups shrink near the end to cut the tail.

HW notes (paired A/B steady-state measurements, repeat-loop NEFFs):
- pack5 ~123-136us/rep vs pack6 baseline; PE streams ~0.46-0.52 ns/row on
  this part (model: 0.4167), DMA measured ~389 GB/s/core (not binding).
- ldweights=False tap-major reuse (pack5r), drain-splitting Act/DVE
  (pack5s), and even removing drains entirely (pack5mm) all measured SLOWER
  than this structure on HW - the emission order here is load-bearing.
- 1024-wide matmuls over 2 PSUM banks (pack5w) and an fp8-e4m3 DoubleRow
  matmul for the single tap (pack5q, would cut PE rows 10%) both pass bass
  but crash this neuronx-cc build's walrus backend; plain For_i vs
  staggered_reset is a wash; xpool bufs 3->4 (this variant) gains
  ~3-5us/rep.

Second-session notes (same-batch interleaved A/B, so same device state):
- The cost-model timeline sim puts this kernel at 110.8us/conv steady
  (PE busy 96%; floor 5*12768*4*0.4167 = 106.4us). HW measures 130-146us.
- The HW-vs-model gap is NOT: Ldweights (pack5t halves weight loads via
  block-pair slot-major emission -> no change), DVE/SBUF-port pressure
  from the B build (pack5b ships B from HBM, DVE idle -> ~+1.5us), out-DMA
  queueing on SP (pack5o untested on HW but SP queue is only ~60% busy),
  or For_i boundaries (pack5x32 measured ~+2us vs x16 - k=16 is the peak).
- The dominant residual is POWER/CLOCK THROTTLING under sustained full-PE
  load, fast (~50-100ms) timescale: per-conv rate is ~116-137us just after
  idle and climbs to ~146us after ~100ms of back-to-back convs; estimates
  from r_big=256 runs ~127-132us vs ~140-148us from r_big=1024 with the
  identical kernel. Structural variants all land within +-2us of each
  other once device state is controlled for.
- pack5q's crash reproduces in this environment (JaxRuntimeError INTERNAL
  "CallFunctionObjArgs" at NEFF compile during the first execution).
- PE rate probe (For_i loop of bare 512-row K=128 matmuls, paired
  differencing): fp16 0.514 ns/row, plain fp8-e4m3 0.435 ns/row (fp16's
  2 B/elem moving-operand fetch is SBUF-bandwidth-bound; fp8 halves it).
  DoublePixel and DoubleColumn COMPILE AND RUN on this toolchain (unlike
  DoubleRow) but are rate-identical to plain fp8 - not throughput modes
  here. 1024-wide PSUM matmul re-confirmed as a walrus crash.
- pack5d<k> (slot 4 as plain fp8, exact rel err 1.25e-2 on this seed,
  validated in sim and on HW) measured a DEAD TIE with pack5x16 in
  interleaved A/B (132982 vs 132969 ns) despite the predicted ~4us PE
  saving - consistent with the power-throttle ceiling, not PE instruction
  time, setting the sustained rate. Shipped variant stays pack5x16_fp16
  (35x more error margin at identical speed).

Third-session PE microbenchmarks (conv-exact 5-matmul accumulation groups,
For_i loops, paired differencing - these supersede the bare-matmul rates
above, which were distorted by per-matmul PSUM bank switching):
- 5 fp16 MMs accumulating into ONE bank: 0.4304 ns/row. The same matmuls
  each into a FRESH bank: 0.514. A 3+2 split across two banks: 0.458.
  => PSUM BANK SWITCHING is the expensive event (~40ns/switch); RMW
  accumulation on one bank is nearly free. This explains pack5r/pack5s/C
  regressions and why 8-deep block-major emission is optimal.
- 4 fp16 + 1 fp8 group: 0.4138 ns/row - the fp8 gain is real in isolation
  but does NOT show up in the full kernel (pack5d tie).
- Conv-exact PE floor = 255360 rows * 0.4304 = 109.9us vs ~130-133us
  measured for the full kernel in the same device state: ~20us is spent
  outside the pure matmul stream and is INSENSITIVE to drain engine
  (pack5s +6.7us), out-DMA queue (pack5o16 +1.8us), B sourcing, weight
  loads, and tap dtype - most likely distributed sequencer/sem-chain
  overhead plus power throttling. All variant families have now been
  HW-tested under interleaved same-state A/B; pack5x16 is the optimum of
  every tested dimension.
- Weight-slot probe (same accum-group structure): 5 DISTINCT lhsT slots
  cost 0.4212 ns/row vs 0.4153 with one slot repeated - weight cycling is
  ~6ns/matmul, i.e. ~1.5us/conv, NOT the gap. ldweights=False on mms 1-4
  measured 0.4366 - SLOWER than just reloading; the reuse path is a
  pessimization in this toolchain (fully explains the pack5r/pack5t
  losses; do not retry weight-reuse schemes here). Remaining gap to the
  PE floor is distributed per-block sem waits / drain-trigger updates /
  p-state dips, none individually dominant; attacking it would need
  Tile-framework-level sync coarsening, not kernel-level emission changes.
"""

import contextlib

import numpy as np

import concourse.bacc as bacc
import concourse.tile as tile
from concourse import mybir
from concourse.bass_utils import run_bass_kernel_spmd

# Problem constants (hardcoded per harness contract).
B, C, H, W = 32, 64, 112, 112
OC, KH, KW = 128, 3, 3
NCORES = 8
BPC = B // NCORES          # images per core
HP, WP = H + 2, W + 2      # host-padded input height/width (114)
XFLAT = HP * WP            # 12996 flat padded-input elements per channel
OFLAT = H * WP             # 12768 flat padded-output elements per channel
BLK = 512                  # matmul free-dim block (= 1 PSUM bank of fp32)
NBLK = (OFLAT + BLK - 1) // BLK  # 25 blocks (24 full + 1 of 480)
XBUF = XFLAT + 4           # SBUF image stride (matmul offsets read to XFLAT+1)
GS = 8                     # out-DMA grouping: 8 blocks per transfer
# Emit the output in fp16 (DVE casts during the PSUM->SBUF drain; the host
# upcasts to fp32). Halves the dominant out-DMA traffic for ~2^-11 extra
# rounding error. Applies to fp16 variants only.
OUT_FP16 = True

# matmul dtype: float32r streams fp32 through the PE at 1 cycle/row for
# free-dim >= 256 (vs 4 cycles/row for plain float32).
MM_DTYPE = mybir.dt.float32r

_cache = {}

# Variant switch (test harness flips this to isolate bottlenecks):
#   "pack6k128_fp16" (default) - fp16 operands, 6 all-K=128 MMs per block:
#        3 tap-pair MMs (taps (0,d)+(1,d) via the one-row-shifted copy on
#        partitions 64-127) + 3 "half-pair" MMs ([0; w(2,d)] at offset WP+d).
#        Measured ~2.9e-4 rel err; K=64 matmuls hit a much slower walrus
#        path, hence all-K=128.
#   "pack6"        - fp32r: 3 K=128 tap-pair MMs + 3 K=64 single MMs per block
#   "pack6_bf16"   - same structure as pack6, bf16 operands
#   "pack6k128_bf16" - as default but bf16
#   "wsplit9_bf16" - bf16 x on both partition halves; lhsT packs [w_hi; w_lo]
#                    per tap (w ~ w_hi + w_lo, near-fp32 weight precision);
#                    9 K=128 MMs per block
#   "wsplit9_ldw"  - wsplit9 tap-major with ldweights=False reuse (slower)
#   "mm_only" / "dma_only" - bottleneck-isolation probes
#   "pack5_fp16"   - fp16, 5 all-K=128 MMs per block: tile A = [x; x>>WP]
#        (host) pairs taps (0,d)+(1,d) at offsets j0+d and the padded single
#        [0; w(2,1)] at j0+WP+1; tile B = [x; x>>2] (built on-device from A
#        by two same-partition DVE copies) pairs taps (2,0)+(2,2) at
#        j0+2*WP. PSUM drains run on the Activation engine.
#   "pack5r_fp16"  - pack5 with tap-major weight reuse (ldweights=False)
#        over each out-group: 5 PE weight loads per 8-block group vs 40.
#        Measured ~6-14us/rep SLOWER than pack5 on HW (paired A/B) - the
#        PE overlaps Ldweights fine; bank-cycling order costs extra.
#   "pack5x_fp16"  - pack5 with xpool bufs=4 (deeper input prefetch);
#        measured ~3-5us/rep faster than pack5 (paired A/B).
#   "pack5x<k>_fp16" (k = 2/4/8/16/32) - pack5x whose repeat-timing builds
#        emit k convs per hardware-loop iteration (divides the per-conv
#        share of the large ~50us+/iteration For_i boundary cost;
#        kernel()'s single-shot program is identical to pack5x). Paired
#        A/B: every doubling won (x2 +3-6, x4 +3-6, x8 +7-9, x16 +5-8
#        us/rep). Timing r values must be divisible by k or the build
#        falls back to one conv per iteration.
#   "pack5h<k>_fp16" - pack5x<k> but tile B is DMA'd straight from the x
#        HBM tensor on the GpSimd (SWDGE) queue: lower = x re-read, upper =
#        x at col offset +2 (bt[C:, j] = x[j+2]). Removes both DVE build
#        copies (DVE goes idle, B becomes SBUF-write-only traffic) at the
#        cost of ~3.3 MB/image extra HBM reads on an otherwise idle queue.
#   "pack5a<k>_fp16" - as pack5h but B DMAs issue on the Activation queue.
#   "pack5t<k>_fp16" - pack5x<k> with tap-major emission over PAIRS of
#        blocks (slot-major: mm(b0,s), mm(b1,s) with ldweights=False on b1):
#        halves PE weight loads (250/conv vs 500) while only alternating
#        between 2 PSUM banks, unlike the 8-bank cycling that made pack5r
#        slower.
VARIANT = "pack5x16_fp16"

import re as _re


def _parse5(variant):
    """-> (family char or '', k) for pack5<fam><k>_fp16 names; (None, 1)
    for non-pack5 or un-batched variants."""
    m = _re.match(r"pack5([a-z]?)(\d*)_fp16$", variant)
    if not m:
        return None, 1
    return m.group(1), int(m.group(2)) if m.group(2) else 1


def loop_batch(variant=None):
    """Convs per hardware-loop iteration for the repeat-timing builds."""
    _, k = _parse5(variant or VARIANT)
    return k


def _build(repeat=1):
    """Build + compile the per-core Bass program (cached per process).

    repeat>1 runs the whole per-core conv `repeat` times back-to-back inside
    one NEFF (idempotent) — used by test.py to measure steady-state device
    time net of dispatch overhead.
    """
    key = ("nc", repeat, VARIANT)
    if key in _cache:
        return _cache[key]
    variant = VARIANT

    nc = bacc.Bacc("TRN2", target_bir_lowering=False, debug=False)
    if variant.endswith("fp16"):
        mm_dt = mybir.dt.float16
    elif variant.endswith("bf16") or variant in ("mm_only", "wsplit9_ldw"):
        mm_dt = mybir.dt.bfloat16
    else:
        mm_dt = MM_DTYPE
    if variant in ("wsplit9_bf16", "mm_only", "wsplit9_ldw"):
        nslot = 9
    elif variant.startswith("pack5"):
        nslot = 5
    else:
        nslot = 6
    fam5, kbatch = _parse5(variant)
    assert variant in (
        "pack6",
        "pack6_bf16",
        "pack6k128_bf16",
        "pack6k128_fp16",
        "pack5_fp16",
        "pack5r_fp16",
        "pack5s_fp16",
        "pack5mm_fp16",
        "pack5w_fp16",
        "pack5p_fp16",
        "pack5q_fp16",
        "wsplit9_bf16",
        "wsplit9_ldw",
        "mm_only",
        "dma_only",
        "dma_only_fp16",
    ) or (
        fam5 in ("x", "h", "a", "t", "b", "o", "d")
        and kbatch in (1, 2, 4, 8, 16, 32, 64)
    ), variant
    # x arrives pre-doubled from the host: per image a [128, XBUF] block whose
    # partitions 0-63 hold the padded image (rows 0-113) and partitions 64-127
    # the same image shifted one row (pack6*) or repeated (wsplit9), so one
    # full-width DMA loads both copies.
    x_ap = nc.dram_tensor(
        "x", [BPC, 2 * C, XBUF], mm_dt, kind="ExternalInput"
    ).ap()
    w_ap = nc.dram_tensor(
        "w", [2 * C, nslot * OC], mm_dt, kind="ExternalInput"
    ).ap()
    out_dt = (
        mybir.dt.float16
        if (OUT_FP16 and mm_dt == mybir.dt.float16)
        else mybir.dt.float32
    )
    out_ap = nc.dram_tensor(
        "out", [BPC, OC, H, WP], out_dt, kind="ExternalOutput"
    ).ap()
    if fam5 == "b":
        # Host-prepped tile B = [x; x>>2]: one full-width 128-partition DMA
        # per image on the GpSimd queue (64-partition DMAs run at half rate).
        xb_ap = nc.dram_tensor(
            "xb", [BPC, 2 * C, XBUF], mm_dt, kind="ExternalInput"
        ).ap()
    if variant == "pack5q_fp16":
        # fp8 path for the single tap (2,1): byte-duplicated e4m3 x (even
        # and odd lanes identical, so the DoubleRow k-subtile convention
        # doesn't matter) and [w8; 0]-interleaved weights on 64 channels.
        x8_ap = nc.dram_tensor(
            "x8", [BPC, C, 2 * XBUF], mybir.dt.float8e4, kind="ExternalInput"
        ).ap()
        w8_ap = nc.dram_tensor(
            "w8", [2 * C, 2 * OC], mybir.dt.float8e4, kind="ExternalInput"
        ).ap()
    if fam5 == "d":
        # fp8 DoublePixel path for the single tap (2,1): x8 upper half is
        # fp8(x) pre-shifted by WP+1 on the host so the slot-4 rhs slice
        # starts at even offset j0 (DoublePixel processes pixel pairs; keep
        # it bank-aligned). Lower half zero, matching [0; w8(2,1)] lhsT.
        x8_ap = nc.dram_tensor(
            "x8", [BPC, 2 * C, XBUF], mybir.dt.float8e4, kind="ExternalInput"
        ).ap()
        w8_ap = nc.dram_tensor(
            "w8", [2 * C, OC], mybir.dt.float8e4, kind="ExternalInput"
        ).ap()

    # pack5w: 1024-wide blocks spanning 2 PSUM banks (half the matmul and
    # drain instruction count; same rows).
    BLKW = 1024 if variant == "pack5w_fp16" else BLK
    NBLKW = (OFLAT + BLKW - 1) // BLKW
    GSW = (GS * BLK) // BLKW  # keep the out-group byte width constant

    with tile.TileContext(nc) as tc:
        with (
            tc.tile_pool(
                name="xpool",
                bufs=4 if fam5 in ("x", "h", "a", "t") else 3,
            ) as xpool,
            tc.tile_pool(name="wpool", bufs=1) as wpool,
            tc.tile_pool(
                name="opool",
                bufs=2 if (variant == "pack5q_fp16" or fam5 == "d") else 4,
            ) as opool,
            tc.tile_pool(
                name="psum",
                bufs=8 * BLK // BLKW,
                space="PSUM",
            ) as pspool,
        ):
            # Weight slots: pack6* = 3 tap-pair slots (K=128: rows 0-63 =
            # tap (0,d), rows 64-127 = tap (1,d)) + 3 single slots (K=64:
            # tap (2,d)); wsplit9 = 9 taps x [w_hi; w_lo].
            wt = wpool.tile([2 * C, nslot * OC], mm_dt)
            # Issue the (tiny) weight DMA from the Activation queue so the
            # first x chunk's DMA issues immediately on the SP queue.
            nc.sync.dma_start(wt[:], w_ap[:])
            if variant == "pack5q_fp16":
                w8t = wpool.tile([2 * C, 2 * OC], mybir.dt.float8e4)
                nc.sync.dma_start(w8t[:], w8_ap[:])
            if fam5 == "d":
                w8t = wpool.tile([2 * C, OC], mybir.dt.float8e4)
                nc.sync.dma_start(w8t[:], w8_ap[:])

            # Column-chunk edges for the x DMA + B build: lets the first
            # blocks' matmuls start after ~1/4 of the image has landed
            # instead of waiting for the full 3.3 MB transfer.
            XCH = [0, 512, 1792, 3840, 6656, 9984, XBUF]

            def conv_pass_pack5(bpool, p5pool=None, d8pool=None):
                for im in range(BPC):
                    xt = xpool.tile([2 * C, XBUF], mm_dt)
                    if p5pool is not None:
                        p5t = p5pool.tile([2 * C, 2 * XBUF], mybir.dt.float8e4)
                    if d8pool is not None:
                        x8t = d8pool.tile([2 * C, XBUF], mybir.dt.float8e4)
                    for ci, (c0, c1) in enumerate(zip(XCH, XCH[1:])):
                        nc.sync.dma_start(xt[:, c0:c1], x_ap[im][:, c0:c1])
                        if d8pool is not None:
                            # fp8 x on the Activation DGE queue: keeps SP free
                            # for the fp16 chunks that gate the first matmuls.
                            nc.scalar.dma_start(
                                x8t[:, c0:c1], x8_ap[im][:, c0:c1]
                            )
                        if p5pool is not None and ci < 2:
                            d0, d1 = [(0, 2 * 6656), (2 * 6656, 2 * XBUF)][ci]
                            nc.sync.dma_start(
                                p5t[:C, d0:d1], x8_ap[im][:, d0:d1]
                            )
                    # B lower = x (copy of A lower). B upper = x>>2: built by
                    # shifting A upper (x>>WP) by +W=112 (DVE variants; x's
                    # top padding row is zero and MM4 only reads B from col
                    # 2*WP on, so the unwritten first W cols of B upper are
                    # never consumed), or DMA'd from the x HBM tensor at col
                    # offset +2 (h/a variants, no DVE work).
                    bt = bpool.tile([2 * C, XBUF], mm_dt)
                    if fam5 == "b":
                        for c0, c1 in ((0, 1792), (1792, 6656), (6656, XBUF)):
                            nc.gpsimd.dma_start(
                                bt[:, c0:c1], xb_ap[im][:, c0:c1]
                            )
                    elif fam5 in ("h", "a"):
                        beng = nc.gpsimd if fam5 == "h" else nc.scalar
                        for c0, c1 in ((0, 1792), (1792, 6656), (6656, XBUF)):
                            beng.dma_start(bt[:C, c0:c1], x_ap[im][:C, c0:c1])
                            u1 = min(c1, XBUF - 2)
                            beng.dma_start(
                                bt[C:, c0:u1], x_ap[im][:C, c0 + 2 : u1 + 2]
                            )
                    else:
                        for c0, c1 in zip(XCH, XCH[1:]):
                            l1 = min(c1, XFLAT)
                            if c0 < l1:
                                nc.vector.tensor_copy(
                                    bt[:C, c0:l1], xt[:C, c0:l1]
                                )
                            u1 = min(c1, XFLAT - W)
                            if c0 < u1:
                                nc.vector.tensor_copy(
                                    bt[C:, W + c0 : W + u1], xt[C:, c0:u1]
                                )
                    o_im = out_ap[im].rearrange("o h w -> o (h w)")
                    # Finer out-DMA groups at the end of the last image so the
                    # final drain+DMA tail is short.
                    if BLKW == 1024:
                        if im < BPC - 1:
                            gedges = [0, 4, 8, 12, 13]
                        else:
                            gedges = [0, 4, 8, 10, 11, 12, 13]
                    elif im < BPC - 1:
                        gedges = [0, 8, 16, 24, 25]
                    else:
                        gedges = [0, 8, 16, 20, 22, 23, 24, 25]
                    # Slots 0-2: taps (0,d)+(1,d) via A at j0+d; slot 3:
                    # (2,0)+(2,2) via B at j0+2*WP; slot 4: [0; w(2,1)] via A
                    # upper at j0+WP+1.
                    def slot_rhs(s, j0, n):
                        if s < 3:
                            return xt[:, j0 + s : j0 + s + n]
                        if s == 3:
                            return bt[:, j0 + 2 * WP : j0 + 2 * WP + n]
                        return xt[:, j0 + WP + 1 : j0 + WP + 1 + n]

                    for gs_, ge_ in zip(gedges, gedges[1:]):
                        ot = opool.tile([OC, GS * BLK], out_dt)
                        g0 = gs_ * BLKW
                        blks = [
                            (blk * BLKW, min(BLKW, OFLAT - blk * BLKW))
                            for blk in range(gs_, ge_)
                        ]
                        if variant == "pack5r_fp16":
                            # Tap-major over the group: one weight load per
                            # slot; the other matmuls reuse the loaded
                            # weights (PE executes in FIFO program order).
                            pss = [
                                pspool.tile(
                                    [OC, BLKW],
                                    mybir.dt.float32,
                                    name=f"ps{bi}",
                                    tag="ps",
                                )
                                for bi in range(len(blks))
                            ]
                            for s in range(5):
                                for bi, (j0, n) in enumerate(blks):
                                    mm = nc.tensor.matmul(
                                        pss[bi][:, :n],
                                        lhsT=wt[:, s * OC : (s + 1) * OC],
                                        rhs=slot_rhs(s, j0, n),
                                        start=(s == 0),
                                        stop=(s == 4),
                                    )
                                    if bi > 0:
                                        mm.ldweights = False
                                    if s == 4:
                                        nc.scalar.copy(
                                            ot[:, bi * BLKW : bi * BLKW + n],
                                            pss[bi][:, :n],
                                        )
                        elif fam5 == "t":
                            # Slot-major over PAIRS of blocks: the second
                            # block's matmul reuses the loaded weights
                            # (ldweights=False) -> half the PE weight loads,
                            # while only alternating between 2 PSUM banks.
                            for p0 in range(0, len(blks), 2):
                                pair = blks[p0 : p0 + 2]
                                pss = [
                                    pspool.tile(
                                        [OC, BLKW],
                                        mybir.dt.float32,
                                        name=f"ps{p0 + pi}",
                                        tag="ps",
                                    )
                                    for pi in range(len(pair))
                                ]
                                for s in range(5):
                                    for pi, (j0, n) in enumerate(pair):
                                        mm = nc.tensor.matmul(
                                            pss[pi][:, :n],
                                            lhsT=wt[:, s * OC : (s + 1) * OC],
                                            rhs=slot_rhs(s, j0, n),
                                            start=(s == 0),
                                            stop=(s == 4),
                                        )
                                        if pi > 0:
                                            mm.ldweights = False
                                        if s == 4:
                                            bi = p0 + pi
                                            nc.scalar.copy(
                                                ot[:, bi * BLKW : bi * BLKW + n],
                                                pss[pi][:, :n],
                                            )
                        else:
                            for bi, (j0, n) in enumerate(blks):
                                ps = pspool.tile([OC, BLKW], mybir.dt.float32)
                                extra8 = p5pool is not None or d8pool is not None
                                nmm = 4 if extra8 else 5
                                for s in range(nmm):
                                    nc.tensor.matmul(
                                        ps[:, :n],
                                        lhsT=wt[:, s * OC : (s + 1) * OC],
                                        rhs=slot_rhs(s, j0, n),
                                        start=(s == 0),
                                        stop=(s == nmm - 1 and not extra8),
                                    )
                                if d8pool is not None:
                                    # Tap (2,1) as a plain fp8 matmul: fp8
                                    # streams 0.435 ns/row vs fp16's 0.514 on
                                    # this part (the fp16 moving-operand
                                    # fetch is SBUF-bandwidth-bound; fp8
                                    # halves it). DoublePixel/DoubleColumn
                                    # measured rate-identical to plain fp8,
                                    # so no perf_mode. x8 upper is
                                    # pre-shifted by WP+1 on the host, so
                                    # the rhs starts at offset j0.
                                    nc.tensor.matmul(
                                        ps[:, :n],
                                        lhsT=w8t[:],
                                        rhs=x8t[:, j0 : j0 + n],
                                        start=False,
                                        stop=True,
                                    )
                                if p5pool is not None:
                                    # Tap (2,1) in fp8 DoubleRow: half the
                                    # PE row cost; odd-lane weights are zero
                                    # so only the duplicated even lanes of
                                    # p5t contribute.
                                    d0 = j0 + 2 * WP + 1
                                    nc.tensor.matmul(
                                        ps[:, :n],
                                        lhsT=w8t[:, : 2 * OC].rearrange(
                                            "k (m two) -> k two m", two=2
                                        ),
                                        rhs=p5t[
                                            :, 2 * d0 : 2 * (d0 + n)
                                        ].rearrange(
                                            "k (n two) -> k two n", two=2
                                        ),
                                        start=False,
                                        stop=True,
                                        perf_mode=mybir.MatmulPerfMode.DoubleRow,
                                    )
                                if variant == "pack5mm_fp16":
                                    continue  # PE-rate probe: no drains
                                # pack5s: alternate drains across Act/DVE
                                eng = (
                                    nc.vector
                                    if (variant == "pack5s_fp16" and bi % 2)
                                    else nc.scalar
                                )
                                if eng is nc.scalar:
                                    eng.copy(
                                        ot[:, bi * BLKW : bi * BLKW + n],
                                        ps[:, :n],
                                    )
                                else:
                                    eng.tensor_copy(
                                        ot[:, bi * BLKW : bi * BLKW + n],
                                        ps[:, :n],
                                    )
                        if variant == "pack5mm_fp16":
                            continue
                        j0, n = blks[-1]
                        gn = j0 + n - g0
                        # 'o': out rides the Activation DGE queue so output
                        # traffic never queues behind the next image's x
                        # chunks on SP.
                        oeng = nc.scalar if fam5 == "o" else nc.sync
                        oeng.dma_start(o_im[:, g0 : g0 + gn], ot[:, :gn])

            def conv_pass():
                for im in range(BPC):
                    # Partitions 0-63: padded image (rows 0-113).
                    # Partitions 64-127: same image shifted one row (+WP), so
                    # a K=128 matmul at offset j0+d contracts taps (0,d) and
                    # (1,d) simultaneously.
                    xt = xpool.tile([2 * C, XBUF], mm_dt)
                    nc.sync.dma_start(xt[:], x_ap[im])
                    o_im = out_ap[im].rearrange("o h w -> o (h w)")

                    if variant == "wsplit9_ldw":
                        # Tap-major over groups of GS blocks: one weight load
                        # per tap per group; the other GS-1 matmuls reuse the
                        # loaded weights (ldweights=False). PE instructions
                        # execute in FIFO program order, so the pairing holds.
                        for g0 in range(0, OFLAT, GS * BLK):
                            blks = [
                                (j0, min(BLK, OFLAT - j0))
                                for j0 in range(g0, min(g0 + GS * BLK, OFLAT), BLK)
                            ]
                            pss = [
                                pspool.tile(
                                    [OC, BLKW],
                                    mybir.dt.float32,
                                    name=f"ps{bi}",
                                    tag="ps",
                                )
                                for bi in range(len(blks))
                            ]
                            for t in range(KH * KW):
                                dh, dw = divmod(t, KW)
                                for bi, (j0, n) in enumerate(blks):
                                    off = j0 + dh * WP + dw
                                    mm = nc.tensor.matmul(
                                        pss[bi][:, :n],
                                        lhsT=wt[:, t * OC : (t + 1) * OC],
                                        rhs=xt[:, off : off + n],
                                        start=(t == 0),
                                        stop=(t == KH * KW - 1),
                                    )
                                    if bi > 0:
                                        mm.ldweights = False
                            ot = opool.tile([OC, GS * BLK], out_dt)
                            for bi, (j0, n) in enumerate(blks):
                                nc.vector.tensor_copy(
                                    ot[:, bi * BLK : bi * BLK + n], pss[bi][:, :n]
                                )
                            gn = blks[-1][0] + blks[-1][1] - g0
                            nc.sync.dma_start(o_im[:, g0 : g0 + gn], ot[:, :gn])
                        continue

                    ot = None
                    for blk in range(NBLK):
                        j0 = blk * BLK
                        n = min(BLK, OFLAT - j0)
                        g = blk % GS
                        if g == 0:
                            ot = opool.tile([OC, GS * BLK], out_dt)
                            g0 = j0
                        if variant.startswith("dma_only"):
                            nc.vector.tensor_copy(
                                ot[:, g * BLK : g * BLK + n], xt[:OC, j0 : j0 + n]
                            )
                        elif variant in ("wsplit9_bf16", "mm_only"):
                            ps = pspool.tile([OC, BLK], mybir.dt.float32)
                            for t in range(KH * KW):
                                dh, dw = divmod(t, KW)
                                off = j0 + dh * WP + dw
                                nc.tensor.matmul(
                                    ps[:, :n],
                                    lhsT=wt[:, t * OC : (t + 1) * OC],
                                    rhs=xt[:, off : off + n],
                                    start=(t == 0),
                                    stop=(t == KH * KW - 1),
                                )
                            if variant == "mm_only":
                                continue
                            nc.vector.tensor_copy(
                                ot[:, g * BLK : g * BLK + n], ps[:, :n]
                            )
                        else:
                            ps = pspool.tile([OC, BLK], mybir.dt.float32)
                            k128 = variant.startswith("pack6k128")
                            for d in range(3):
                                nc.tensor.matmul(
                                    ps[:, :n],
                                    lhsT=wt[:, d * OC : (d + 1) * OC],
                                    rhs=xt[:, j0 + d : j0 + d + n],
                                    start=(d == 0),
                                    stop=False,
                                )
                            for d in range(3):
                                if k128:
                                    # Slot 3+d = [0; w(2,d)]: upper half (copy
                                    # B, +WP shift) contributes tap (2,d) at
                                    # offset WP+d; lower half is zeroed.
                                    nc.tensor.matmul(
                                        ps[:, :n],
                                        lhsT=wt[:, (3 + d) * OC : (4 + d) * OC],
                                        rhs=xt[:, j0 + WP + d : j0 + WP + d + n],
                                        start=False,
                                        stop=(d == 2),
                                    )
                                else:
                                    nc.tensor.matmul(
                                        ps[:, :n],
                                        lhsT=wt[:C, (3 + d) * OC : (4 + d) * OC],
                                        rhs=xt[
                                            :C,
                                            j0 + 2 * WP + d : j0 + 2 * WP + d + n,
                                        ],
                                        start=False,
                                        stop=(d == 2),
                                    )
                            nc.vector.tensor_copy(
                                ot[:, g * BLK : g * BLK + n], ps[:, :n]
                            )
                        if g == GS - 1 or blk == NBLK - 1:
                            gn = j0 + n - g0
                            nc.sync.dma_start(
                                o_im[:, g0 : g0 + gn], ot[:, :gn]
                            )

            # repeat < 0: unrolled -repeat passes (timeline-sim analysis only;
            # the sim can't execute the register-mode hardware loop).
            if variant.startswith("pack5"):
                with contextlib.ExitStack() as stack:
                    bpool = stack.enter_context(
                        tc.tile_pool(name="bpool", bufs=2)
                    )
                    p5pool = (
                        stack.enter_context(tc.tile_pool(name="p5pool", bufs=2))
                        if variant == "pack5q_fp16"
                        else None
                    )
                    d8pool = (
                        stack.enter_context(tc.tile_pool(name="d8pool", bufs=2))
                        if fam5 == "d"
                        else None
                    )
                    if repeat == 1:
                        conv_pass_pack5(bpool, p5pool, d8pool)
                    elif repeat < 0:
                        for _ in range(-repeat):
                            conv_pass_pack5(bpool, p5pool, d8pool)
                    elif kbatch > 1 and repeat % kbatch == 0:
                        # k convs per hardware-loop iteration: divides the
                        # per-conv share of the loop-boundary cost by k.
                        with tc.For_i(0, repeat // kbatch, 1, staggered_reset=True):
                            for _ in range(kbatch):
                                conv_pass_pack5(bpool, p5pool, d8pool)
                    else:
                        with tc.For_i(
                            0,
                            repeat,
                            1,
                            staggered_reset=(variant != "pack5p_fp16"),
                        ):
                            conv_pass_pack5(bpool, p5pool, d8pool)
            elif repeat == 1:
                conv_pass()
            elif repeat < 0:
                for _ in range(-repeat):
                    conv_pass()
            else:
                with tc.For_i(0, repeat, 1):
                    conv_pass()

    nc.compile()
    _cache[key] = nc
    return nc


def run_on_device(nc, in_maps):
    """Single-exec jitted runner with device-resident inputs; returns a
    callable for repeated timing plus the output fetcher."""
    from jax.sharding import Mesh, NamedSharding, PartitionSpec
    from jax.experimental.shard_map import shard_map
    import jax

    from concourse.bass2jax import (
        _bass_exec_p,
        install_neuronx_cc_hook,
        partition_id_tensor,
    )

    install_neuronx_cc_hook()

    partition_name = nc.partition_id_tensor.name if nc.partition_id_tensor else None
    in_names, out_names, out_avals = [], [], []
    for alloc in nc.m.functions[0].allocations:
        if not isinstance(alloc, mybir.MemoryLocationSet):
            continue
        name = alloc.memorylocations[0].name
        if alloc.kind == "ExternalInput":
            if name != partition_name:
                in_names.append(name)
        elif alloc.kind == "ExternalOutput":
            out_names.append(name)
            out_avals.append(
                jax.core.ShapedArray(
                    tuple(alloc.tensor_shape), mybir.dt.np(alloc.dtype)
                )
            )
    n_params = len(in_names)
    all_in_names = list(in_names) + list(out_names)
    if partition_name is not None:
        all_in_names.append(partition_name)
    all_in_names = tuple(all_in_names)

    def body(*args):
        operands = list(args)
        if partition_name is not None:
            operands.append(partition_id_tensor())
        return tuple(
            _bass_exec_p.bind(
                *operands,
                out_avals=tuple(out_avals),
                in_names=all_in_names,
                out_names=tuple(out_names),
                lowering_input_output_aliases=(),
                sim_require_finite=True,
                sim_require_nnan=True,
                nc=nc,
            )
        )

    n_cores = len(in_maps)
    devices = jax.devices()[:n_cores]
    mesh = Mesh(np.asarray(devices), ("core",))
    nspecs = n_params + len(out_names)
    sharded = jax.jit(
        shard_map(
            body,
            mesh=mesh,
            in_specs=(PartitionSpec("core"),) * nspecs,
            out_specs=(PartitionSpec("core"),) * len(out_names),
            check_rep=False,
        )
    )
    concat_in = [
        np.concatenate([np.asarray(in_maps[c][nm]) for c in range(n_cores)], axis=0)
        for nm in in_names
    ]
    concat_zeros = [
        np.zeros((n_cores * a.shape[0], *a.shape[1:]), a.dtype) for a in out_avals
    ]
    sharding = NamedSharding(mesh, PartitionSpec("core"))
    dev_in = [jax.device_put(a, sharding) for a in concat_in]
    dev_zeros = [jax.device_put(a, sharding) for a in concat_zeros]

    def run():
        return sharded(*dev_in, *dev_zeros)

    return run, out_names, out_avals


def _prep_inputs(x, weight):
    """Host-side shard + layout prep. Returns per-core input maps."""
    import ml_dtypes

    variant = VARIANT
    if variant.endswith("fp16"):
        host_dt = np.float16
    elif variant.endswith("bf16") or variant in ("mm_only", "wsplit9_ldw"):
        host_dt = ml_dtypes.bfloat16
    else:
        host_dt = np.float32

    xp = np.zeros((B, C, HP, WP), dtype=np.float32)
    xp[:, :, 1 : H + 1, 1 : W + 1] = x
    flat = xp.reshape(B, C, XFLAT).astype(host_dt)
    xprep = np.zeros((B, 2 * C, XBUF), dtype=host_dt)
    xprep[:, :C, :XFLAT] = flat
    if variant in ("wsplit9_bf16", "mm_only", "wsplit9_ldw"):
        xprep[:, C:, :XFLAT] = flat
    else:
        xprep[:, C:, : XFLAT - WP] = flat[:, :, WP:]

    w4 = weight[0]  # (out_c, in_c, kh, kw)
    if variant.startswith("pack5"):
        # Slots d=0..2: rows 0-63 = w(0,d), rows 64-127 = w(1,d) (pairs via
        # tile A). Slot 3: rows 0-63 = w(2,0), rows 64-127 = w(2,2) (pair via
        # tile B = [x; x>>2]). Slot 4: rows 0-63 = 0, rows 64-127 = w(2,1)
        # (single via A upper at offset WP+1).
        wp = np.zeros((2 * C, 5, OC), dtype=host_dt)
        for d in range(KW):
            wp[:C, d] = w4[:, :, 0, d].T.astype(host_dt)
            wp[C:, d] = w4[:, :, 1, d].T.astype(host_dt)
        wp[:C, 3] = w4[:, :, 2, 0].T.astype(host_dt)
        wp[C:, 3] = w4[:, :, 2, 2].T.astype(host_dt)
        wp[C:, 4] = w4[:, :, 2, 1].T.astype(host_dt)
        w_prep = np.ascontiguousarray(wp.reshape(2 * C, 5 * OC))
    elif variant in ("wsplit9_bf16", "mm_only", "wsplit9_ldw"):
        # lhsT slot t: rows 0-63 = bf16(w[tap t]), rows 64-127 = bf16 of the
        # residual -> contraction over both halves gives ~fp32 weight
        # precision at bf16 matmul rate.
        w_hi = w4.astype(ml_dtypes.bfloat16)
        w_lo = (w4.astype(np.float32) - w_hi.astype(np.float32)).astype(
            ml_dtypes.bfloat16
        )
        wp = np.zeros((2 * C, KH * KW, OC), dtype=host_dt)
        for t in range(KH * KW):
            kh, kw = divmod(t, KW)
            wp[:C, t] = w_hi[:, :, kh, kw].T
            wp[C:, t] = w_lo[:, :, kh, kw].T
        w_prep = np.ascontiguousarray(wp.reshape(2 * C, KH * KW * OC))
    else:
        # lhsT slots: pairs d=0..2 pack taps (0,d) [rows 0-63] + (1,d)
        # [rows 64-127]. Singles 3+d hold tap (2,d): in rows 0-63 for the
        # K=64 variant, in rows 64-127 (zero top, used with the +WP-shifted
        # copy at offset WP+d) for the all-K=128 variant.
        wp = np.zeros((2 * C, 6, OC), dtype=host_dt)
        for d in range(KW):
            wp[:C, d] = w4[:, :, 0, d].T.astype(host_dt)
            wp[C:, d] = w4[:, :, 1, d].T.astype(host_dt)
            if variant.startswith("pack6k128"):
                wp[C:, 3 + d] = w4[:, :, 2, d].T.astype(host_dt)
            else:
                wp[:C, 3 + d] = w4[:, :, 2, d].T.astype(host_dt)
        w_prep = np.ascontiguousarray(wp.reshape(2 * C, 6 * OC))
    if variant == "pack5q_fp16":
        f8 = mybir.dt.np(mybir.dt.float8e4)
        x8 = np.asarray(flat, dtype=np.float32).astype(f8)  # [B, C, XFLAT]
        x8d = np.zeros((B, C, 2 * XBUF), dtype=f8)
        x8d[:, :, 0 : 2 * XFLAT : 2] = x8
        x8d[:, :, 1 : 2 * XFLAT : 2] = x8
        w8p = np.zeros((2 * C, 2 * OC), dtype=f8)
        w8p[:C, 0::2] = w4[:, :, 2, 1].T.astype(np.float32).astype(f8)
        return [
            {
                "x": xprep[c * BPC : (c + 1) * BPC],
                "w": w_prep,
                "x8": x8d[c * BPC : (c + 1) * BPC],
                "w8": w8p,
            }
            for c in range(NCORES)
        ]
    fam5, _ = _parse5(variant)
    if fam5 == "d":
        f8 = mybir.dt.np(mybir.dt.float8e4)
        # Upper = fp8(x) pre-shifted by 2*WP+1 (tap (2,1) lands at rhs
        # offset j0); lower stays zero to match the [0; w8(2,1)] lhsT.
        x8p = np.zeros((B, 2 * C, XBUF), dtype=f8)
        x8p[:, C:, : XFLAT - 2 * WP - 1] = flat[:, :, 2 * WP + 1 :].astype(f8)
        w8p = np.zeros((2 * C, OC), dtype=f8)
        w8p[C:] = w4[:, :, 2, 1].T.astype(host_dt).astype(f8)
        return [
            {
                "x": xprep[c * BPC : (c + 1) * BPC],
                "w": w_prep,
                "x8": x8p[c * BPC : (c + 1) * BPC],
                "w8": w8p,
            }
            for c in range(NCORES)
        ]
    if fam5 == "b":
        # Tile B = [x; x>>2] shipped whole: lower = padded image, upper =
        # same image shifted 2 cols (tap (2,2) alignment at offset 2*WP).
        xbprep = np.zeros((B, 2 * C, XBUF), dtype=host_dt)
        xbprep[:, :C, :XFLAT] = flat
        xbprep[:, C:, : XFLAT - 2] = flat[:, :, 2:]
        return [
            {
                "x": xprep[c * BPC : (c + 1) * BPC],
                "w": w_prep,
                "xb": xbprep[c * BPC : (c + 1) * BPC],
            }
            for c in range(NCORES)
        ]
    return [
        {"x": xprep[c * BPC : (c + 1) * BPC], "w": w_prep} for c in range(NCORES)
    ]


def kernel(x, weight):
    x = np.asarray(x, dtype=np.float32)
    weight = np.asarray(weight, dtype=np.float32)
    # Rare nondeterministic output corruption was observed on this part (one
    # single-shot execution of a long-validated build returned rel err 4.4e-2
    # while the same build measured 3.6e-4 before and after). Two defenses:
    # run the idempotent 16-pass loop program (every pass rewrites the full
    # output, so a transient flake in any non-final pass self-heals), and
    # execute it twice, accepting only a bitwise-agreeing result.
    nc = _build(repeat=loop_batch()) if loop_batch() > 1 else _build()
    in_maps = _prep_inputs(x, weight)
    prev = None
    for attempt in range(4):
        res = run_bass_kernel_spmd(nc, in_maps, list(range(NCORES)))
        out = np.concatenate(
            [res.results[c]["out"] for c in range(NCORES)], axis=0
        )
        if prev is not None and np.array_equal(out, prev):
            break
        prev = out
    return np.ascontiguousarray(out[:, :, :, :W].astype(np.float32))

